# revision 22
# baseline (speedup 1.0000x reference)
"""Equivariant rotation conv for Trainium2, 8-core batch-parallel.

Computes: rotate a (128*8, 128, 3, 3) filter bank by 8 data-dependent angles
(bilinear resampling), run a 3x3 same-padded conv of x (16,128,128,128) with
all 8*128 rotated filters, then max over the 8 rotations -> (16,128,128,128).

Sharding: data-parallel over batch, 2 images per core; the filter bank and
rotation coefficients are replicated.

The bilinear rotation of the filter bank is a (rot_alpha-dependent) 9x9
mixing matrix per rotation applied to the 3x3 taps - ~10 MFLOP against the
conv's ~620 GFLOP - so it runs on the host, which ships pre-quantized
bf16/fp8 rotated filter banks.  On device, per core:
  - mixed-precision conv: rotations {0,2} (the sharpest filters, which win
    the rotation-max on most pixels) run as 9 shifted bf16 PE matmuls per
    psum tile; rotations {1,3,4,5,6,7} run in fp8-e4m3 DoubleRow perf mode,
    pairing the 8 off-center taps into 4 K=256 matmuls at 2x MAC rate plus
    the center tap in bf16 - 5 PE slots instead of 9.  Rel error of the
    final max stays ~1.7e-2 (< 2e-2) because fp8 quantization noise only
    lands on the ~45% of pixels whose argmax is a smoothed rotation,
  - the DoubleRow rhs pair dim is materialized as two fp8 copies of the x
    block: xpair (halves = x shifted 0 / +2 cols) covers tap pairs (0,2),
    (3,5), (6,8); xqair (halves = x shifted 0 / +2 rows, at kx=1) covers
    pair (1,7); lhsT pairs are step-slices of the tap dim of the fp8
    filter tile,
  - a running elementwise max over the rotation chunks on DVE (the only
    non-PE engine that can read PSUM), with the final max fused with the
    per-slice output DMA; the accumulator and output are bf16 (upcast on
    host), halving store traffic,
  - a uniform pipelined loop over 8 row blocks: f32 staging is 2-deep,
    cast operand tiles 3-deep; ~125 dummy matmuls warm the PE clock up to
    2.4 GHz before the first real group.
"""

import numpy as np
import ml_dtypes


def _install_axon_hooks_shim():
    """Provide antenv.axon_hooks (NTFF profile hook) when the image's antenv
    lacks it, so run_bass_kernel_spmd(trace=True) works instead of crashing
    on import.  The hook drives NRT profiling via ctypes into the axon PJRT
    plugin, mirroring the boot-side installer."""
    import contextlib
    import ctypes
    import os
    import sys
    import types

    try:
        import antenv.axon_hooks  # noqa: F401

        return
    except ImportError:
        pass

    state = {"hook": None, "resolved": False}

    def _make_hook():
        so_path = os.environ.get("AXON_PJRT_SO", "/opt/axon/libaxon_pjrt.so")
        if not os.path.exists(so_path):
            return None
        lib = ctypes.CDLL(so_path)
        if not hasattr(lib, "axon_start_nrt_profile"):
            return None
        lib.axon_start_nrt_profile.argtypes = [
            ctypes.POINTER(ctypes.c_int64),
            ctypes.c_size_t,
        ]
        lib.axon_start_nrt_profile.restype = ctypes.c_int64
        lib.axon_stop_nrt_profile.argtypes = [ctypes.c_char_p]
        lib.axon_stop_nrt_profile.restype = ctypes.c_int64

        @contextlib.contextmanager
        def _hook(output_dir, device_ids):
            import jax

            jax.devices()
            if device_ids:
                ids = (ctypes.c_int64 * len(device_ids))(*device_ids)
                rc = lib.axon_start_nrt_profile(ids, len(device_ids))
            else:
                rc = lib.axon_start_nrt_profile(None, 0)
            if rc != 0:
                raise RuntimeError(f"axon_start_nrt_profile rc={rc}")
            try:
                yield
            finally:
                n = lib.axon_stop_nrt_profile(str(output_dir).encode())
                if n < 0:
                    raise RuntimeError(f"axon_stop_nrt_profile rc={n}")
                print(f"profile: {n} file(s) written to {output_dir}")

        return _hook

    mod = types.ModuleType("antenv.axon_hooks")

    def set_axon_ntff_profile_hook(h):
        state["hook"] = h
        state["resolved"] = True

    def get_axon_ntff_profile_hook():
        if not state["resolved"]:
            state["hook"] = _make_hook()
            state["resolved"] = True
        return state["hook"]

    mod.set_axon_ntff_profile_hook = set_axon_ntff_profile_hook
    mod.get_axon_ntff_profile_hook = get_axon_ntff_profile_hook
    sys.modules["antenv.axon_hooks"] = mod


_install_axon_hooks_shim()

import concourse.bass as bass
import concourse.mybir as mybir
from concourse import bacc
from concourse.bass_utils import run_bass_kernel_spmd
from concourse.tile import TileContext

F32 = mybir.dt.float32
BF16 = mybir.dt.bfloat16
FP8 = mybir.dt.float8e4
DR = mybir.MatmulPerfMode.DoubleRow

B, CIN, H, W = 16, 128, 128, 128
R, O, K = 8, 128, 3
NCORES = 8
BL = B // NCORES  # images per core
RB = 32           # output rows per block
NS = RB // 4      # psum subtiles (4 rows = 512 cols) per block
NBLK = H // RB

# Rotations computed in fp8 DoubleRow (4 paired taps + bf16 center tap);
# the rest run fully in bf16.  {0,2} have the least bilinear smoothing,
# win the max most often, and so carry most of the accuracy budget.
BF16_ROT = (0, 2)
FP8_ROT = (1, 3, 4, 5, 6, 7)
# rotation 2 is "mixed": its two lowest-energy expressible tap pairs run as
# fp8 DoubleRow, the remaining 5 taps stay bf16 (7 PE slots instead of 9);
# its win share (~17%) is high enough that full fp8 would break the 2e-2
# error gate (sim: 2.0e-2) while this partial split sims at 1.88e-2
MIX_ROT = 2
MIX_PAIRS = ((0, 2), (6, 8))
MIX_BF16_TAPS = (1, 3, 4, 5, 7)
W8_ROT = (1, 2, 3, 4, 5, 6, 7)   # rotations shipping an fp8 bank
W8_IDX = {r: i for i, r in enumerate(W8_ROT)}
FP8_IDX = {r: i for i, r in enumerate(FP8_ROT)}
BF16_IDX = {r: i for i, r in enumerate(BF16_ROT)}
# Off-center tap pairs for DoubleRow: (0,2),(3,5),(6,8) pair kx=0 with
# kx=2 at fixed ky (rhs = xpair); (1,7) pairs ky=0 with ky=2 at kx=1
# (rhs = xqair).  Tap 4 (center) runs as a single bf16 matmul.
TAP_PAIRS = ((0, 2), (3, 5), (6, 8), (1, 7))

MODE = "hybrid-hostrot"

_TRACE = False
LAST_RESULTS = None
_NC_CACHE = {}


def _rot_mats(rot_alpha):
    """Per-rotation 9x9 bilinear resampling matrices, matching the reference
    F.grid_sample(align_corners=True, zeros) tap logic exactly.

    M[r, p, q]: coefficient of original tap q = (qy*3+qx) in rotated tap
    p = (py*3+px)."""
    M = np.zeros((R, 9, 9), np.float64)
    lin = np.linspace(-1.0, 1.0, K)
    for r in range(R):
        ang = float(rot_alpha[r]) * (np.pi / 4.0) * r
        c, s = np.cos(ang), np.sin(ang)
        for a in range(K):          # output row (gy = lin[a])
            for b in range(K):      # output col (gx = lin[b])
                gx, gy = lin[b], lin[a]
                xs = c * gx - s * gy
                ys = s * gx + c * gy
                ix = (xs + 1.0) * 0.5 * (K - 1)
                iy = (ys + 1.0) * 0.5 * (K - 1)
                x0 = int(np.floor(ix))
                y0 = int(np.floor(iy))
                wx, wy = ix - x0, iy - y0
                p = a * K + b
                for yi, xi, wt in (
                    (y0, x0, (1 - wy) * (1 - wx)),
                    (y0, x0 + 1, (1 - wy) * wx),
                    (y0 + 1, x0, wy * (1 - wx)),
                    (y0 + 1, x0 + 1, wy * wx),
                ):
                    if 0 <= yi < K and 0 <= xi < K:
                        M[r, p, yi * K + xi] += wt
    return M.astype(np.float32)


def _build(mode):
    assert mode == "hybrid-hostrot"

    nc = bacc.Bacc(trn_type="TRN2")
    xs = nc.dram_tensor("xs", [BL, CIN, H, W], F32, kind="ExternalInput")
    # host-rotated, host-quantized filter banks (layout [i=cin, p=tap, o])
    wbf = nc.dram_tensor("wbf", [len(BF16_ROT), CIN, 9, O], BF16,
                         kind="ExternalInput")
    w8 = nc.dram_tensor("w8", [len(W8_ROT), CIN, 9, O], FP8,
                        kind="ExternalInput")
    wc4 = nc.dram_tensor("wc4", [len(FP8_ROT), CIN, O], BF16,
                         kind="ExternalInput")
    # bf16 output (upcast on host): halves the store DMA traffic and lets
    # the accumulator live in bf16.
    y = nc.dram_tensor("y", [BL, O, H, W], BF16, kind="ExternalOutput")

    with TileContext(nc) as tc:
        with (
            tc.tile_pool(name="wrot", bufs=1) as rpool,
            tc.tile_pool(name="xio", bufs=1) as xpool,
            tc.tile_pool(name="accp", bufs=3) as apool,
            tc.tile_pool(name="psum", bufs=1, space="PSUM") as ppool,
        ):
            rotb = {r: rpool.tile([128, 9, O], BF16, name=f"rb_{r}",
                                  tag=f"rb_{r}") for r in BF16_ROT}
            rotc4 = {r: rpool.tile([128, O], BF16, name=f"rc4_{r}",
                                   tag=f"rc4_{r}") for r in FP8_ROT}
            rot8 = {r: rpool.tile([128, 9, O], FP8, name=f"r8_{r}",
                                  tag=f"r8_{r}") for r in W8_ROT}

            # PE warm-up: ~125 dependency-free matmuls on a zeroed scratch
            # tile keep the PE busy from ~0.5us until the first real matmul,
            # so the HAM clock gate reaches 8/8 before real work and the
            # first conv chunks run at 2.4 GHz instead of 1.2.
            dum_lhs = rpool.tile([128, 128], BF16, name="dum_lhs", tag="dum")
            nc.vector.memset(dum_lhs[:, :], 0.0)
            dum_ps = ppool.tile([128, 128], F32, name="dum_ps", tag="ps0")
            for _ in range(78):
                nc.tensor.matmul(
                    dum_ps[:, :], dum_lhs[:, :], dum_lhs[:, :],
                    start=True, stop=True,
                )

            # x staging: manual ping-pong between two persistent f32 buffers
            # so the zero padding (columns 0 and W+1, boundary halo rows) is
            # established once instead of re-memset every block.  Each block
            # is then cast into three matmul-operand tiles:
            #   xbf   [34, 130] bf16 - all taps of bf16 rotations + center
            #   xpair [2, 34, 128] fp8 - halves at kx=0 / kx=2
            #   xqair [2, 32, 128] fp8 - halves at ky=0 / ky=2, kx=1
            xst2 = [
                xpool.tile([128, RB + 2, W + 2], F32, name=f"xst{i}", tag=f"xst{i}")
                for i in range(2)
            ]
            xbf2 = [
                xpool.tile([128, RB + 2, W + 2], BF16, name=f"xbf{i}", tag=f"xbf{i}")
                for i in range(3)
            ]
            xpr2 = [
                xpool.tile([128, 2, RB + 2, W], FP8, name=f"xpr{i}", tag=f"xpr{i}")
                for i in range(3)
            ]
            xqr2 = [
                xpool.tile([128, 2, RB, W], FP8, name=f"xqr{i}", tag=f"xqr{i}")
                for i in range(3)
            ]
            for i in range(2):
                nc.gpsimd.memset(xst2[i][:, :, :], 0.0)

            def load_x(g, b, blk, chunks=1, cuts=None, first_cast_dve=False,
                       w0_after_first_chunk=False):
                # DMA the block's input rows (with halo) into the ping-pong
                # staging buffer, then cast into the three operand tiles.
                # `cuts` splits the load so downstream matmuls can start on
                # the first rows before the whole block has landed.
                h0 = blk * RB
                r0 = max(h0 - 1, 0)
                r1 = min(h0 + RB + 1, H)
                xst = xst2[g % 2]
                xbf = xbf2[g % 3]
                xpr = xpr2[g % 3]
                xqr = xqr2[g % 3]
                if g >= 2:
                    # restore halo-row zeros clobbered by the previous user
                    # of this buffer (interior blocks write all 34 rows)
                    if blk == 0:
                        nc.gpsimd.memset(xst[:, 0:1, :], 0.0)
                    elif blk == NBLK - 1:
                        nc.gpsimd.memset(xst[:, RB + 1 : RB + 2, :], 0.0)
                d0 = r0 - (h0 - 1)
                nrows = r1 - r0
                if cuts is None:
                    cuts = [nrows * k // chunks for k in range(chunks + 1)]
                for k in range(len(cuts) - 1):
                    a, c = cuts[k], cuts[k + 1]
                    nc.sync.dma_start(
                        out=xst[:, d0 + a : d0 + c, 1 : W + 1],
                        in_=xs[b, :, r0 + a : r0 + c, :],
                    )
                    # cast range covers the pad rows on the outer chunks
                    ca = d0 + a if k > 0 else 0
                    cc = d0 + c if k < len(cuts) - 2 else RB + 2
                    if k == 0 and first_cast_dve:  # noqa: SIM114
                        # first chunk cast on DVE, so the PE can start early
                        nc.vector.tensor_copy(
                            xbf[:, ca:cc, :], xst[:, ca:cc, :]
                        )
                        if w0_after_first_chunk:
                            # rotation 0's filters ride the DMA queue right
                            # behind the first x rows: both gate the first
                            # real matmul group
                            nc.sync.dma_start(
                                out=rotb[0][:, :, :], in_=wbf[0, :, :, :]
                            )
                    else:
                        nc.scalar.copy(xbf[:, ca:cc, :], xst[:, ca:cc, :])
                # fp8 operand tiles (consumed from rotation 1 on)
                nc.scalar.copy(xpr[:, 0, :, :], xst[:, :, 0:W])
                nc.scalar.copy(xpr[:, 1, :, :], xst[:, :, 2 : W + 2])
                nc.scalar.copy(xqr[:, 0, :, :], xst[:, 0:RB, 1 : W + 1])
                nc.scalar.copy(xqr[:, 1, :, :], xst[:, 2 : RB + 2, 1 : W + 1])
                return xbf, xpr, xqr

            def conv_chunk(xt, acc, r, store=None, s_groups=1):
                xbf, xpr, xqr = xt
                pst = [
                    ppool.tile([128, 4, W], F32, name=f"ps{s}", tag=f"ps{s}")
                    for s in range(NS)
                ]
                is8 = r in FP8_IDX

                def emit_dr_pair(s, pa, pb, first):
                    lhsT = rot8[r][:, pa : pb + 1 : pb - pa, :]
                    if pa in (0, 3, 6):         # (ky fixed, kx 0&2)
                        ky = pa // 3
                        rhs = xpr[:, :, 4 * s + ky : 4 * s + ky + 4, :]
                    else:                       # (1,7): ky 0&2, kx=1
                        rhs = xqr[:, :, 4 * s : 4 * s + 4, :]
                    nc.tensor.matmul(
                        pst[s][:, :, :], lhsT, rhs,
                        start=first, stop=False, perf_mode=DR,
                    )

                def emit_group(ss):
                    for s in ss:
                        if is8:
                            for j, (pa, pb) in enumerate(TAP_PAIRS):
                                emit_dr_pair(s, pa, pb, j == 0)
                            nc.tensor.matmul(
                                pst[s][:, :, :], rotc4[r][:, :],
                                xbf[:, 4 * s + 1 : 4 * s + 5, 1 : 1 + W],
                                start=False, stop=True,
                            )
                        elif r == MIX_ROT:
                            for j, (pa, pb) in enumerate(MIX_PAIRS):
                                emit_dr_pair(s, pa, pb, j == 0)
                            for n, p in enumerate(MIX_BF16_TAPS):
                                ky, kx = divmod(p, 3)
                                nc.tensor.matmul(
                                    pst[s][:, :, :], rotb[r][:, p, :],
                                    xbf[:, 4 * s + ky : 4 * s + ky + 4, kx : kx + W],
                                    start=False,
                                    stop=(n == len(MIX_BF16_TAPS) - 1),
                                )
                        else:
                            for p in range(9):
                                ky, kx = divmod(p, 3)
                                nc.tensor.matmul(
                                    pst[s][:, :, :], rotb[r][:, p, :],
                                    xbf[:, 4 * s + ky : 4 * s + ky + 4, kx : kx + W],
                                    start=(p == 0), stop=(p == 8),
                                )
                    for s in ss:
                        # DVE is the only non-PE engine that can read PSUM,
                        # so the running max lives there
                        if r == 0:
                            nc.vector.tensor_copy(
                                acc[:, 4 * s : 4 * s + 4, :], pst[s][:, :, :]
                            )
                        else:
                            nc.vector.tensor_tensor(
                                acc[:, 4 * s : 4 * s + 4, :],
                                acc[:, 4 * s : 4 * s + 4, :],
                                pst[s][:, :, :],
                                mybir.AluOpType.max,
                            )
                        if store is not None:
                            b, h0 = store
                            nc.sync.dma_start(
                                out=y[b, :, h0 + 4 * s : h0 + 4 * s + 4, :],
                                in_=acc[:, 4 * s : 4 * s + 4, :],
                            )

                per = NS // s_groups
                for k in range(s_groups):
                    emit_group(range(k * per, (k + 1) * per))

            # DMA issue order (the sync queue issues serially): rotation 0's
            # filters and the first 5 x rows go first so the first matmul
            # group is unblocked as early as possible; the remaining filter
            # banks follow, then the later x blocks.
            xt0 = load_x(0, 0, 0, cuts=[0, 3, 5, 14, 23, 33],
                         first_cast_dve=True, w0_after_first_chunk=True)
            for r in FP8_ROT:
                nc.sync.dma_start(out=rot8[r][:, :, :], in_=w8[W8_IDX[r], :, :, :])
                nc.sync.dma_start(out=rotc4[r][:, :], in_=wc4[FP8_IDX[r], :, :])
            nc.sync.dma_start(out=rotb[2][:, :, :], in_=wbf[1, :, :, :])
            nc.sync.dma_start(
                out=rot8[MIX_ROT][:, :, :], in_=w8[W8_IDX[MIX_ROT], :, :, :]
            )

            last_g = BL * NBLK - 1
            xt = xt0
            for g in range(BL * NBLK):
                b, blk = divmod(g, NBLK)
                if g > 0:
                    xt = load_x(g, b, blk, chunks=2 if g < 3 else 1)
                acc = apool.tile([128, RB, W], BF16, name=f"acc{g}", tag="acc")
                for r in range(R):
                    final = r == R - 1
                    conv_chunk(
                        xt, acc, r,
                        store=(b, blk * RB) if final else None,
                        s_groups=8 if (final and g == last_g) else 1,
                    )
    nc.finalize()
    return nc


def _get_nc():
    if MODE not in _NC_CACHE:
        _NC_CACHE[MODE] = _build(MODE)
    return _NC_CACHE[MODE]


def kernel(x, weight, rot_alpha):
    global LAST_RESULTS
    x = np.ascontiguousarray(np.asarray(x, np.float32))
    weight = np.ascontiguousarray(np.asarray(weight, np.float32))
    rot_alpha = np.asarray(rot_alpha, np.float32)

    # host-side filter rotation: rot[r, i, p, o] = sum_q M[r,p,q] w[o*R+r, i, q]
    M = _rot_mats(rot_alpha)
    wq = np.ascontiguousarray(
        weight.reshape(O, R, CIN, 9).transpose(1, 2, 3, 0)
    )  # [r, i, q, o]
    rot = np.einsum("rpq,riqo->ripo", M, wq)  # [r, i, p, o] f32
    wbf = np.ascontiguousarray(rot[list(BF16_ROT)]).astype(ml_dtypes.bfloat16)
    w8 = np.ascontiguousarray(rot[list(W8_ROT)]).astype(ml_dtypes.float8_e4m3)
    wc4 = np.ascontiguousarray(rot[list(FP8_ROT), :, 4, :]).astype(
        ml_dtypes.bfloat16
    )

    nc = _get_nc()
    in_maps = [
        {
            "xs": np.ascontiguousarray(x[c * BL : (c + 1) * BL]),
            "wbf": wbf,
            "w8": w8,
            "wc4": wc4,
        }
        for c in range(NCORES)
    ]
    try:
        res = run_bass_kernel_spmd(nc, in_maps, list(range(NCORES)), trace=_TRACE)
    except Exception:
        # One retry (without tracing): a failed compile or an aborted run can
        # leave a NeuronCore transiently wedged; the next attempt recovers.
        res = run_bass_kernel_spmd(nc, in_maps, list(range(NCORES)), trace=False)
    LAST_RESULTS = res
    out = np.concatenate(
        [np.asarray(res.results[c]["y"]) for c in range(NCORES)], axis=0
    )
    return out.astype(np.float32)
